# revision 36
# baseline (speedup 1.0000x reference)
"""CantorExpert MoE-routing kernel for 8 Trainium2 NeuronCores.

Strategy
--------
Host (numpy, cheap):
  - fingerprint-band mask + token gather (data-dependent indexing, same as
    the reference does on host), zero-pad selected tokens to a multiple of
    128, pre-transpose to feature-major layout.
  - fold the normalized pentachoron directions into the QKV weights
    (feats @ (W @ dirs.T) == (feats @ W) @ dirs.T), so the 15 per-vertex
    projections become one extra N=16 matmul chunk.
  - alpha-sigmoid scalars.

Device (Bass/Tile, one batch element per core; B=8 == n_cores):
  - gate MLP: feats @ ag_w1 -> gelu -> (*ag_w2, reduce) -> sigmoid -> vis
  - Q/K/V:    feats @ [wq|wk|wv] accumulated over K=1024 in PSUM,
              scaled by vis on the PSUM->SBUF copy (scaling commutes with
              the contraction since vis is per-token).
  - proj:     feats @ folded_dirs, scaled by vis.

The visibility scale is applied to the matmul *outputs* (per-token scalar
broadcast along the free dim), which is algebraically identical to scaling
feats before the matmuls but avoids a feature-major rescale pass.
"""

import os
from contextlib import ExitStack

import numpy as np

EXPERT_ID = 3
NUM_EXPERTS = 8
FULL_DIM = 8192
EXPERT_DIM = 2048
SLICE = FULL_DIM // NUM_EXPERTS  # 1024
SL_START = EXPERT_ID * SLICE
SL_END = SL_START + SLICE
FP_MIN = EXPERT_ID / NUM_EXPERTS
FP_MAX = (EXPERT_ID + 1) / NUM_EXPERTS
B = 8
N_CORES = 8
KT = SLICE // 128  # 8 k-tiles over the contraction dim
CH = 512  # qkv free-dim chunk (one fp32 PSUM bank)
NQKV = 3 * EXPERT_DIM  # 6144

# matmul operand dtype: float16 (1 cycle/row on the PE like bf16, but with a
# 10-bit mantissa), overridable for accuracy/perf experiments.
DT_NAME = os.environ.get("KERNEL_DT", "float16")

_nc_cache = {}
LAST_RESULTS = None


def _split_waits(nc, limit=1):
    """This walrus build accepts only one sync-wait per instruction; hoist
    extras onto preceding same-engine NOPs."""
    import bass_rust
    import concourse.mybir as mybir

    for fn in nc.m.functions:
        for blk in fn.blocks:
            out = []
            for inst in blk.instructions:
                si = inst.sync_info
                if si is not None and len(si.on_wait) > limit:
                    waits = list(si.on_wait)
                    extra, keep = waits[:-limit], waits[-limit:]
                    for j, w in enumerate(extra):
                        nop = mybir.InstNoOp(
                            name=f"{inst.name}-wsplit{j}", ins=[], outs=[]
                        )
                        nop.engine = inst.engine
                        nop.sync_info = bass_rust.SyncInfo(on_wait=[w], on_update=[])
                        out.append(nop)
                    inst.sync_info = bass_rust.SyncInfo(
                        on_wait=keep, on_update=list(si.on_update)
                    )
                out.append(inst)
            blk.instructions = out


def _strip_epilogue(nc):
    """Keep only the global SP drain in Tile's end block — the two all-engine
    barrier waves + sem clears after it cost ~1-2us and are only needed for
    sem hygiene, which NRT re-establishes per execution (verified by running
    the NEFF twice in-process)."""
    blk = nc.m.functions[0].blocks[-1]
    if not blk.name.endswith("_end"):
        return
    insts = blk.instructions
    for j, inst in enumerate(insts):
        op = inst.opcode if hasattr(inst, "opcode") else ""
        if op == "Drain" and "SP" in str(inst.engine):
            blk.instructions = insts[: j + 1]
            return


def _build_nc(Tp, dt_name, add_b1):
    import concourse.bass as bass
    import concourse.mybir as mybir
    import concourse.tile as tile

    DT = getattr(mybir.dt, dt_name)
    f32 = mybir.dt.float32
    MT = Tp // 128
    act = mybir.ActivationFunctionType
    alu = mybir.AluOpType

    nc = bass.Bass("TRN2", target_bir_lowering=False, debug=False,
                   num_devices=N_CORES)

    featsT = nc.dram_tensor("featsT", [128, KT, Tp], DT, kind="ExternalInput")
    # weights pre-chunked on host so every DMA reads contiguous per-partition
    # runs (strided slices of one big W tensor drop DMA to ~1/3 efficiency)
    wgate = nc.dram_tensor("wgate", [128, KT, 256], DT, kind="ExternalInput")
    wproj = nc.dram_tensor("wproj", [128, KT, 16], DT, kind="ExternalInput")
    wqkv = nc.dram_tensor(
        "wqkv", [NQKV // CH, 128, KT, CH], DT, kind="ExternalInput"
    )
    w2b = nc.dram_tensor("w2b", [128, 256], f32, kind="ExternalInput")
    b2aw = nc.dram_tensor("b2aw", [128, 3], f32, kind="ExternalInput")
    b1b = (
        nc.dram_tensor("b1b", [128, 256], f32, kind="ExternalInput")
        if add_b1
        else None
    )
    # outputs in [p, m, n] layout (token t == m*128 + p) so each chunk's
    # result leaves in ONE DMA; host untangles during unshard
    q_out = nc.dram_tensor("q_out", [128, MT, EXPERT_DIM], f32,
                           kind="ExternalOutput")
    k_out = nc.dram_tensor("k_out", [128, MT, EXPERT_DIM], f32,
                           kind="ExternalOutput")
    v_out = nc.dram_tensor("v_out", [128, MT, EXPERT_DIM], f32,
                           kind="ExternalOutput")
    p_out = nc.dram_tensor("p_out", [128, MT, 16], f32, kind="ExternalOutput")
    qkv_outs = [q_out, k_out, v_out]

    with ExitStack() as ctx:
        tc = ctx.enter_context(tile.TileContext(nc))
        consts = ctx.enter_context(tc.tile_pool(name="consts", bufs=1))
        fpool = ctx.enter_context(tc.tile_pool(name="fpool", bufs=1))
        wpool = ctx.enter_context(tc.tile_pool(name="wpool", bufs=6))
        gpool = ctx.enter_context(tc.tile_pool(name="gpool", bufs=4))
        opool = ctx.enter_context(tc.tile_pool(name="opool", bufs=3))
        pspool = ctx.enter_context(tc.tile_pool(name="pspool", bufs=8, space="PSUM"))

        # ---- input staging ----
        # Each DMA trigger costs ~650ns of issuing-engine time and Tile
        # tracks tile dependencies at whole-tile granularity, so: feats in
        # two tiles (halves of k) split across the SP and ACT HWDGE rings,
        # gate weights first on SP, consts afterwards. The first matmul can
        # then start ~3.5us in instead of ~9us.
        # ---- PE warmup ----
        # The PE clock-gate (HAM) starts at half rate and needs ~3.4us of
        # sustained activity to release. Burn that during the ~7us input-DMA
        # front with dummy matmuls on a zeroed scratch tile so the real
        # stream runs at full rate from its first instruction.
        if int(os.environ.get("KERNEL_WARMUP", "0")):
            warm = fpool.tile([128, CH], DT, tag="warm")
            nc.vector.memset(warm, 0.0)
            wps = pspool.tile([128, CH], f32, tag="ps")
            for _ in range(12):
                nc.tensor.matmul(wps, warm[:, :128], warm, start=True, stop=True)

        KH = KT // 2
        wg = wpool.tile([128, KT, 256], DT, tag="wg")
        nc.sync.dma_start(out=wg, in_=wgate[:, :, :])
        ft_a = fpool.tile([128, KH, Tp], DT, tag="fta")
        nc.scalar.dma_start(out=ft_a, in_=featsT[:, :KH, :])
        ft_b = fpool.tile([128, KT - KH, Tp], DT, tag="ftb")
        nc.sync.dma_start(out=ft_b, in_=featsT[:, KH:, :])

        def ft(k):
            return ft_a[:, k, :] if k < KH else ft_b[:, k - KH, :]

        w2b_t = consts.tile([128, 256], f32, tag="w2b")
        nc.sync.dma_start(out=w2b_t, in_=w2b[:, :])
        b2aw_t = consts.tile([128, 3], f32, tag="b2aw")
        nc.sync.dma_start(out=b2aw_t, in_=b2aw[:, :])
        if add_b1:
            b1b_t = consts.tile([128, 256], f32, tag="b1b")
            nc.sync.dma_start(out=b1b_t, in_=b1b[:, :])
        vis = consts.tile([128, MT], f32, tag="vis")

        # ---- gate MLP -> per-token visibility scale ----
        for m in range(MT):
            ms = slice(m * 128, (m + 1) * 128)
            ps = pspool.tile([128, 256], f32, tag="ps")
            for k in range(KT):
                nc.tensor.matmul(
                    ps, ft(k)[:, ms], wg[:, k, :],
                    start=(k == 0), stop=(k == KT - 1),
                )
            if add_b1:
                nc.vector.tensor_tensor(out=ps, in0=ps, in1=b1b_t, op=alu.add)
            h = gpool.tile([128, 256], f32, tag="h")
            nc.scalar.activation(out=h, in_=ps, func=act.Gelu)
            hw = gpool.tile([128, 256], f32, tag="hw")
            gs = gpool.tile([128, 1], f32, tag="gs")
            nc.vector.tensor_tensor(out=hw, in0=h, in1=w2b_t, op=alu.mult)
            nc.vector.reduce_sum(out=gs, in_=hw, axis=mybir.AxisListType.X)
            sg = gpool.tile([128, 1], f32, tag="sg")
            nc.scalar.activation(
                out=sg, in_=gs, func=act.Sigmoid, bias=b2aw_t[:, 0:1], scale=1.0
            )
            nc.vector.tensor_scalar(
                out=vis[:, m:m + 1], in0=sg,
                scalar1=b2aw_t[:, 1:2], scalar2=b2aw_t[:, 2:3],
                op0=alu.mult, op1=alu.add,
            )

        # ---- Q/K/V chunks ----
        for c in range(NQKV // CH):
            wt = wpool.tile([128, KT, CH], DT, tag="w")
            nc.scalar.dma_start(out=wt, in_=wqkv[c, :, :, :])
            dst = qkv_outs[c // 4]
            coff = (c % 4) * CH
            ob = opool.tile([128, MT, CH], f32, tag="ob")
            for m in range(MT):
                ms = slice(m * 128, (m + 1) * 128)
                ps = pspool.tile([128, CH], f32, tag="ps")
                for k in range(KT):
                    nc.tensor.matmul(
                        ps, ft(k)[:, ms], wt[:, k, :],
                        start=(k == 0), stop=(k == KT - 1),
                    )
                nc.vector.tensor_scalar_mul(
                    out=ob[:, m, :], in0=ps, scalar1=vis[:, m:m + 1]
                )
                if c == NQKV // CH - 1:
                    # last chunk: ship each m-tile as soon as its copy lands
                    # so the final drain only waits on one small transfer
                    nc.sync.dma_start(
                        out=dst[:, m, coff:coff + CH], in_=ob[:, m, :]
                    )
            if c != NQKV // CH - 1:
                # alternate HWDGE rings to balance W-in vs results-out traffic
                eng = nc.sync if c % 2 == 0 else nc.scalar
                eng.dma_start(out=dst[:, :, coff:coff + CH], in_=ob)

        # ---- pentachoron projections (dirs folded into weights) ----
        wt = wpool.tile([128, KT, 16], DT, tag="wg")
        nc.scalar.dma_start(out=wt, in_=wproj[:, :, :])
        obp = opool.tile([128, MT, 16], f32, tag="obp")
        for m in range(MT):
            ms = slice(m * 128, (m + 1) * 128)
            ps = pspool.tile([128, 16], f32, tag="ps")
            for k in range(KT):
                nc.tensor.matmul(
                    ps, ft(k)[:, ms], wt[:, k, :],
                    start=(k == 0), stop=(k == KT - 1),
                )
            nc.vector.tensor_scalar_mul(
                out=obp[:, m, :], in0=ps, scalar1=vis[:, m:m + 1]
            )
        nc.sync.dma_start(out=p_out[:, :, :], in_=obp)

    _split_waits(nc)
    _strip_epilogue(nc)
    return nc


def kernel(tokens, fingerprints, alpha, ag_w1, ag_b1, ag_w2, ag_b2,
           wq, wk, wv, pentachoron):
    import concourse.mybir as mybir
    from concourse.bass_utils import run_bass_kernel_spmd

    tokens = np.asarray(tokens)
    fingerprints = np.asarray(fingerprints, dtype=np.float32)
    alpha = np.float32(np.asarray(alpha))
    ag_w1 = np.asarray(ag_w1, dtype=np.float32)
    ag_b1 = np.asarray(ag_b1, dtype=np.float32)
    ag_w2 = np.asarray(ag_w2, dtype=np.float32)
    ag_b2 = np.asarray(ag_b2, dtype=np.float32)
    wq = np.asarray(wq, dtype=np.float32)
    wk = np.asarray(wk, dtype=np.float32)
    wv = np.asarray(wv, dtype=np.float32)
    pentachoron = np.asarray(pentachoron, dtype=np.float32)

    mask = (fingerprints >= FP_MIN) & (fingerprints < FP_MAX)
    idx = np.nonzero(mask)[0]
    Psel = len(idx)
    Bn = tokens.shape[0]
    if Psel == 0:
        z = np.zeros((Bn, 0, EXPERT_DIM), np.float32)
        return z, z.copy(), z.copy(), np.zeros((3, 5, Bn, 0), np.float32), mask

    Tp = max(128, -(-Psel // 128) * 128)
    MT = Tp // 128

    # gathered expert slice: [B, Psel, SLICE], zero-padded to Tp tokens
    feats = np.zeros((Bn, Tp, SLICE), np.float32)
    feats[:, :Psel, :] = tokens[:, idx, SL_START:SL_END]

    # fold normalized pentachoron dirs into the QKV weights (float64)
    pent64 = pentachoron.astype(np.float64)
    dirs = pent64 / np.maximum(
        np.linalg.norm(pent64, axis=-1, keepdims=True), 1e-12
    )
    pd = np.zeros((SLICE, 16), np.float64)
    # stack order in the reference einsum is [K, Q, V]
    pd[:, 0:5] = wk.astype(np.float64) @ dirs.T
    pd[:, 5:10] = wq.astype(np.float64) @ dirs.T
    pd[:, 10:15] = wv.astype(np.float64) @ dirs.T

    qkv_w = np.concatenate([wq, wk, wv], axis=1)  # [SLICE, NQKV]

    aw = np.float32(1.0 / (1.0 + np.exp(-np.float64(alpha))))
    b2aw = np.broadcast_to(
        np.array([ag_b2[0], aw, np.float32(1.0) - aw], np.float32), (128, 3)
    ).copy()
    w2b = np.broadcast_to(ag_w2[:, 0], (128, 256)).copy()
    add_b1 = bool(np.any(ag_b1))
    b1b = np.broadcast_to(ag_b1, (128, 256)).copy() if add_b1 else None

    np_dt = {"float16": np.float16, "float32r": np.float32,
             "float32": np.float32}.get(DT_NAME)
    if np_dt is None:
        import ml_dtypes
        np_dt = ml_dtypes.bfloat16

    # [row, col] -> [p, k, col] with k*128+p == row; chunk-major for qkv so
    # each chunk DMA reads contiguous per-partition runs
    wgate_dev = np.ascontiguousarray(
        ag_w1.reshape(KT, 128, 256).transpose(1, 0, 2).astype(np_dt)
    )
    wproj_dev = np.ascontiguousarray(
        pd.astype(np.float32).reshape(KT, 128, 16).transpose(1, 0, 2).astype(np_dt)
    )
    wqkv_dev = np.ascontiguousarray(
        qkv_w.reshape(KT, 128, NQKV // CH, CH).transpose(2, 1, 0, 3).astype(np_dt)
    )

    key = (Tp, DT_NAME, add_b1)
    if key not in _nc_cache:
        _nc_cache[key] = _build_nc(Tp, DT_NAME, add_b1)
    nc = _nc_cache[key]

    in_maps = []
    for b in range(N_CORES):
        fb = feats[b % Bn]  # [Tp, SLICE]
        featsT_dev = np.ascontiguousarray(
            fb.T.reshape(KT, 128, Tp).transpose(1, 0, 2)
        ).astype(np_dt)
        m = {"featsT": featsT_dev, "wgate": wgate_dev, "wproj": wproj_dev,
             "wqkv": wqkv_dev, "w2b": w2b, "b2aw": b2aw}
        if add_b1:
            m["b1b"] = b1b
        in_maps.append(m)

    trace = bool(int(os.environ.get("KERNEL_TRACE", "0")))
    kw = {"trace": True, "trace_cores": [0]} if trace else {}
    res = run_bass_kernel_spmd(nc, in_maps, core_ids=list(range(N_CORES)), **kw)
    global LAST_RESULTS
    LAST_RESULTS = res

    Q = np.empty((Bn, Psel, EXPERT_DIM), np.float32)
    K = np.empty((Bn, Psel, EXPERT_DIM), np.float32)
    V = np.empty((Bn, Psel, EXPERT_DIM), np.float32)
    proj = np.empty((3, 5, Bn, Psel), np.float32)
    for b in range(Bn):
        r = res.results[b]
        # device layout is [p, m, n] with token t == m*128 + p
        def tok(a):
            return a.transpose(1, 0, 2).reshape(Tp, -1)[:Psel]

        Q[b] = tok(r["q_out"])
        K[b] = tok(r["k_out"])
        V[b] = tok(r["v_out"])
        proj[:, :, b, :] = tok(r["p_out"])[:, :15].T.reshape(3, 5, Psel)
    return Q, K, V, proj, mask


# revision 39
# speedup vs baseline: 1.1865x; 1.1865x over previous
"""CantorExpert MoE-routing kernel for 8 Trainium2 NeuronCores.

Strategy
--------
Host (numpy, cheap):
  - fingerprint-band mask + token gather (data-dependent indexing, same as
    the reference does on host), zero-pad selected tokens to a multiple of
    128, pre-transpose to feature-major layout.
  - fold the normalized pentachoron directions into the QKV weights
    (feats @ (W @ dirs.T) == (feats @ W) @ dirs.T), so the 15 per-vertex
    projections become one extra N=16 matmul chunk.
  - alpha-sigmoid scalars.

Device (Bass/Tile, one batch element per core; B=8 == n_cores):
  - gate MLP: feats @ ag_w1 -> gelu -> (*ag_w2, reduce) -> sigmoid -> vis
  - Q/K/V:    feats @ [wq|wk|wv] accumulated over K=1024 in PSUM,
              scaled by vis on the PSUM->SBUF copy (scaling commutes with
              the contraction since vis is per-token).
  - proj:     feats @ folded_dirs, scaled by vis.

The visibility scale is applied to the matmul *outputs* (per-token scalar
broadcast along the free dim), which is algebraically identical to scaling
feats before the matmuls but avoids a feature-major rescale pass.
"""

import os
from contextlib import ExitStack

import numpy as np

EXPERT_ID = 3
NUM_EXPERTS = 8
FULL_DIM = 8192
EXPERT_DIM = 2048
SLICE = FULL_DIM // NUM_EXPERTS  # 1024
SL_START = EXPERT_ID * SLICE
SL_END = SL_START + SLICE
FP_MIN = EXPERT_ID / NUM_EXPERTS
FP_MAX = (EXPERT_ID + 1) / NUM_EXPERTS
B = 8
N_CORES = 8
KT = SLICE // 128  # 8 k-tiles over the contraction dim
CH = 512  # qkv free-dim chunk (one fp32 PSUM bank)
NQKV = 3 * EXPERT_DIM  # 6144

# matmul operand dtype: float16 (1 cycle/row on the PE like bf16, but with a
# 10-bit mantissa), overridable for accuracy/perf experiments.
DT_NAME = os.environ.get("KERNEL_DT", "float16")

_nc_cache = {}
LAST_RESULTS = None


def _split_waits(nc, limit=1):
    """This walrus build accepts only one sync-wait per instruction; hoist
    extras onto preceding same-engine NOPs."""
    import bass_rust
    import concourse.mybir as mybir

    for fn in nc.m.functions:
        for blk in fn.blocks:
            out = []
            for inst in blk.instructions:
                si = inst.sync_info
                if si is not None and len(si.on_wait) > limit:
                    waits = list(si.on_wait)
                    extra, keep = waits[:-limit], waits[-limit:]
                    for j, w in enumerate(extra):
                        nop = mybir.InstNoOp(
                            name=f"{inst.name}-wsplit{j}", ins=[], outs=[]
                        )
                        nop.engine = inst.engine
                        nop.sync_info = bass_rust.SyncInfo(on_wait=[w], on_update=[])
                        out.append(nop)
                    inst.sync_info = bass_rust.SyncInfo(
                        on_wait=keep, on_update=list(si.on_update)
                    )
                out.append(inst)
            blk.instructions = out


def _strip_epilogue(nc):
    """Keep only the global SP drain in Tile's end block — the two all-engine
    barrier waves + sem clears after it cost ~1-2us and are only needed for
    sem hygiene, which NRT re-establishes per execution (verified by running
    the NEFF twice in-process)."""
    blk = nc.m.functions[0].blocks[-1]
    if not blk.name.endswith("_end"):
        return
    insts = blk.instructions
    for j, inst in enumerate(insts):
        op = inst.opcode if hasattr(inst, "opcode") else ""
        if op == "Drain" and "SP" in str(inst.engine):
            blk.instructions = insts[: j + 1]
            return


def _build_nc(Tp, dt_name, add_b1):
    import concourse.bass as bass
    import concourse.mybir as mybir
    import concourse.tile as tile

    DT = getattr(mybir.dt, dt_name)
    f32 = mybir.dt.float32
    MT = Tp // 128
    act = mybir.ActivationFunctionType
    alu = mybir.AluOpType

    nc = bass.Bass("TRN2", target_bir_lowering=False, debug=False,
                   num_devices=N_CORES)

    featsT = nc.dram_tensor("featsT", [128, KT, Tp], DT, kind="ExternalInput")
    # weights pre-chunked on host so every DMA reads contiguous per-partition
    # runs (strided slices of one big W tensor drop DMA to ~1/3 efficiency)
    wgate = nc.dram_tensor("wgate", [128, KT, 256], DT, kind="ExternalInput")
    wproj = nc.dram_tensor("wproj", [128, KT, 16], DT, kind="ExternalInput")
    wqkv = nc.dram_tensor(
        "wqkv", [NQKV // CH, 128, KT, CH], DT, kind="ExternalInput"
    )
    w2b = nc.dram_tensor("w2b", [128, 256], f32, kind="ExternalInput")
    b2aw = nc.dram_tensor("b2aw", [128, 3], f32, kind="ExternalInput")
    b1b = (
        nc.dram_tensor("b1b", [128, 256], f32, kind="ExternalInput")
        if add_b1
        else None
    )
    # outputs in [p, m, n] layout (token t == m*128 + p) so each chunk's
    # result leaves in ONE DMA; host untangles during unshard
    q_out = nc.dram_tensor("q_out", [128, MT, EXPERT_DIM], f32,
                           kind="ExternalOutput")
    k_out = nc.dram_tensor("k_out", [128, MT, EXPERT_DIM], f32,
                           kind="ExternalOutput")
    v_out = nc.dram_tensor("v_out", [128, MT, EXPERT_DIM], f32,
                           kind="ExternalOutput")
    p_out = nc.dram_tensor("p_out", [128, MT, 16], f32, kind="ExternalOutput")
    qkv_outs = [q_out, k_out, v_out]

    with ExitStack() as ctx:
        tc = ctx.enter_context(tile.TileContext(nc))
        consts = ctx.enter_context(tc.tile_pool(name="consts", bufs=1))
        fpool = ctx.enter_context(tc.tile_pool(name="fpool", bufs=1))
        wpool = ctx.enter_context(tc.tile_pool(name="wpool", bufs=6))
        gpool = ctx.enter_context(tc.tile_pool(name="gpool", bufs=4))
        opool = ctx.enter_context(tc.tile_pool(name="opool", bufs=3))
        pspool = ctx.enter_context(tc.tile_pool(name="pspool", bufs=8, space="PSUM"))

        # ---- input staging ----
        # Each DMA trigger costs ~650ns of issuing-engine time and Tile
        # tracks tile dependencies at whole-tile granularity, so: feats in
        # two tiles (halves of k) split across the SP and ACT HWDGE rings,
        # gate weights first on SP, consts afterwards. The first matmul can
        # then start ~3.5us in instead of ~9us.
        # ---- PE warmup ----
        # The PE clock-gate (HAM) starts at half rate and needs ~3.4us of
        # sustained activity to release. Burn that during the ~7us input-DMA
        # front with dummy matmuls on a zeroed scratch tile so the real
        # stream runs at full rate from its first instruction.
        if int(os.environ.get("KERNEL_WARMUP", "0")):
            warm = fpool.tile([128, CH], DT, tag="warm")
            nc.vector.memset(warm, 0.0)
            wps = pspool.tile([128, CH], f32, tag="ps")
            for _ in range(12):
                nc.tensor.matmul(wps, warm[:, :128], warm, start=True, stop=True)

        KH = KT // 2
        wg = wpool.tile([128, KT, 256], DT, tag="wg")
        nc.sync.dma_start(out=wg, in_=wgate[:, :, :])
        ft_a = fpool.tile([128, KH, Tp], DT, tag="fta")
        nc.scalar.dma_start(out=ft_a, in_=featsT[:, :KH, :])
        ft_b = fpool.tile([128, KT - KH, Tp], DT, tag="ftb")
        nc.sync.dma_start(out=ft_b, in_=featsT[:, KH:, :])

        def ft(k):
            return ft_a[:, k, :] if k < KH else ft_b[:, k - KH, :]

        w2b_t = consts.tile([128, 256], f32, tag="w2b")
        nc.sync.dma_start(out=w2b_t, in_=w2b[:, :])
        b2aw_t = consts.tile([128, 3], f32, tag="b2aw")
        nc.sync.dma_start(out=b2aw_t, in_=b2aw[:, :])
        if add_b1:
            b1b_t = consts.tile([128, 256], f32, tag="b1b")
            nc.sync.dma_start(out=b1b_t, in_=b1b[:, :])
        vis = consts.tile([128, MT], f32, tag="vis")

        # ---- gate MLP -> per-token visibility scale ----
        for m in range(MT):
            ms = slice(m * 128, (m + 1) * 128)
            ps = pspool.tile([128, 256], f32, tag="ps")
            for k in range(KT):
                nc.tensor.matmul(
                    ps, ft(k)[:, ms], wg[:, k, :],
                    start=(k == 0), stop=(k == KT - 1),
                )
            if add_b1:
                nc.vector.tensor_tensor(out=ps, in0=ps, in1=b1b_t, op=alu.add)
            h = gpool.tile([128, 256], f32, tag="h")
            nc.scalar.activation(out=h, in_=ps, func=act.Gelu)
            hw = gpool.tile([128, 256], f32, tag="hw")
            gs = gpool.tile([128, 1], f32, tag="gs")
            nc.vector.tensor_tensor(out=hw, in0=h, in1=w2b_t, op=alu.mult)
            nc.vector.reduce_sum(out=gs, in_=hw, axis=mybir.AxisListType.X)
            sg = gpool.tile([128, 1], f32, tag="sg")
            nc.scalar.activation(
                out=sg, in_=gs, func=act.Sigmoid, bias=b2aw_t[:, 0:1], scale=1.0
            )
            nc.vector.tensor_scalar(
                out=vis[:, m:m + 1], in0=sg,
                scalar1=b2aw_t[:, 1:2], scalar2=b2aw_t[:, 2:3],
                op0=alu.mult, op1=alu.add,
            )

        # ---- Q/K/V chunks ----
        for c in range(NQKV // CH):
            wt = wpool.tile([128, KT, CH], DT, tag="w")
            nc.scalar.dma_start(out=wt, in_=wqkv[c, :, :, :])
            dst = qkv_outs[c // 4]
            coff = (c % 4) * CH
            ob = opool.tile([128, MT, CH], f32, tag="ob")
            for m in range(MT):
                ms = slice(m * 128, (m + 1) * 128)
                ps = pspool.tile([128, CH], f32, tag="ps")
                for k in range(KT):
                    nc.tensor.matmul(
                        ps, ft(k)[:, ms], wt[:, k, :],
                        start=(k == 0), stop=(k == KT - 1),
                    )
                nc.vector.tensor_scalar_mul(
                    out=ob[:, m, :], in0=ps, scalar1=vis[:, m:m + 1]
                )
                if c == NQKV // CH - 1:
                    # last chunk: ship each m-tile as soon as its copy lands
                    # so the final drain only waits on one small transfer
                    nc.sync.dma_start(
                        out=dst[:, m, coff:coff + CH], in_=ob[:, m, :]
                    )
            if c != NQKV // CH - 1:
                # alternate HWDGE rings to balance W-in vs results-out traffic
                eng = nc.sync if c % 2 == 0 else nc.scalar
                eng.dma_start(out=dst[:, :, coff:coff + CH], in_=ob)

        # ---- pentachoron projections (dirs folded into weights) ----
        wt = wpool.tile([128, KT, 16], DT, tag="wg")
        nc.scalar.dma_start(out=wt, in_=wproj[:, :, :])
        obp = opool.tile([128, MT, 16], f32, tag="obp")
        for m in range(MT):
            ms = slice(m * 128, (m + 1) * 128)
            ps = pspool.tile([128, 16], f32, tag="ps")
            for k in range(KT):
                nc.tensor.matmul(
                    ps, ft(k)[:, ms], wt[:, k, :],
                    start=(k == 0), stop=(k == KT - 1),
                )
            nc.vector.tensor_scalar_mul(
                out=obp[:, m, :], in0=ps, scalar1=vis[:, m:m + 1]
            )
        nc.sync.dma_start(out=p_out[:, :, :], in_=obp)


    _split_waits(nc)
    _strip_epilogue(nc)
    return nc


def kernel(tokens, fingerprints, alpha, ag_w1, ag_b1, ag_w2, ag_b2,
           wq, wk, wv, pentachoron):
    import concourse.mybir as mybir
    from concourse.bass_utils import run_bass_kernel_spmd

    tokens = np.asarray(tokens)
    fingerprints = np.asarray(fingerprints, dtype=np.float32)
    alpha = np.float32(np.asarray(alpha))
    ag_w1 = np.asarray(ag_w1, dtype=np.float32)
    ag_b1 = np.asarray(ag_b1, dtype=np.float32)
    ag_w2 = np.asarray(ag_w2, dtype=np.float32)
    ag_b2 = np.asarray(ag_b2, dtype=np.float32)
    wq = np.asarray(wq, dtype=np.float32)
    wk = np.asarray(wk, dtype=np.float32)
    wv = np.asarray(wv, dtype=np.float32)
    pentachoron = np.asarray(pentachoron, dtype=np.float32)

    mask = (fingerprints >= FP_MIN) & (fingerprints < FP_MAX)
    idx = np.nonzero(mask)[0]
    Psel = len(idx)
    Bn = tokens.shape[0]
    if Psel == 0:
        z = np.zeros((Bn, 0, EXPERT_DIM), np.float32)
        return z, z.copy(), z.copy(), np.zeros((3, 5, Bn, 0), np.float32), mask

    Tp = max(128, -(-Psel // 128) * 128)
    MT = Tp // 128

    # gathered expert slice: [B, Psel, SLICE], zero-padded to Tp tokens
    feats = np.zeros((Bn, Tp, SLICE), np.float32)
    feats[:, :Psel, :] = tokens[:, idx, SL_START:SL_END]

    # fold normalized pentachoron dirs into the QKV weights (float64)
    pent64 = pentachoron.astype(np.float64)
    dirs = pent64 / np.maximum(
        np.linalg.norm(pent64, axis=-1, keepdims=True), 1e-12
    )
    pd = np.zeros((SLICE, 16), np.float64)
    # stack order in the reference einsum is [K, Q, V]
    pd[:, 0:5] = wk.astype(np.float64) @ dirs.T
    pd[:, 5:10] = wq.astype(np.float64) @ dirs.T
    pd[:, 10:15] = wv.astype(np.float64) @ dirs.T

    qkv_w = np.concatenate([wq, wk, wv], axis=1)  # [SLICE, NQKV]

    aw = np.float32(1.0 / (1.0 + np.exp(-np.float64(alpha))))
    b2aw = np.broadcast_to(
        np.array([ag_b2[0], aw, np.float32(1.0) - aw], np.float32), (128, 3)
    ).copy()
    w2b = np.broadcast_to(ag_w2[:, 0], (128, 256)).copy()
    add_b1 = bool(np.any(ag_b1))
    b1b = np.broadcast_to(ag_b1, (128, 256)).copy() if add_b1 else None

    np_dt = {"float16": np.float16, "float32r": np.float32,
             "float32": np.float32}.get(DT_NAME)
    if np_dt is None:
        import ml_dtypes
        np_dt = ml_dtypes.bfloat16

    # [row, col] -> [p, k, col] with k*128+p == row; chunk-major for qkv so
    # each chunk DMA reads contiguous per-partition runs
    wgate_dev = np.ascontiguousarray(
        ag_w1.reshape(KT, 128, 256).transpose(1, 0, 2).astype(np_dt)
    )
    wproj_dev = np.ascontiguousarray(
        pd.astype(np.float32).reshape(KT, 128, 16).transpose(1, 0, 2).astype(np_dt)
    )
    wqkv_dev = np.ascontiguousarray(
        qkv_w.reshape(KT, 128, NQKV // CH, CH).transpose(2, 1, 0, 3).astype(np_dt)
    )

    key = (Tp, DT_NAME, add_b1)
    if key not in _nc_cache:
        _nc_cache[key] = _build_nc(Tp, DT_NAME, add_b1)
    nc = _nc_cache[key]

    in_maps = []
    for b in range(N_CORES):
        fb = feats[b % Bn]  # [Tp, SLICE]
        featsT_dev = np.ascontiguousarray(
            fb.T.reshape(KT, 128, Tp).transpose(1, 0, 2)
        ).astype(np_dt)
        m = {"featsT": featsT_dev, "wgate": wgate_dev, "wproj": wproj_dev,
             "wqkv": wqkv_dev, "w2b": w2b, "b2aw": b2aw}
        if add_b1:
            m["b1b"] = b1b
        in_maps.append(m)

    trace = bool(int(os.environ.get("KERNEL_TRACE", "0")))
    kw = {"trace": True, "trace_cores": [0]} if trace else {}
    res = run_bass_kernel_spmd(nc, in_maps, core_ids=list(range(N_CORES)), **kw)
    global LAST_RESULTS
    LAST_RESULTS = res

    Q = np.empty((Bn, Psel, EXPERT_DIM), np.float32)
    K = np.empty((Bn, Psel, EXPERT_DIM), np.float32)
    V = np.empty((Bn, Psel, EXPERT_DIM), np.float32)
    proj = np.empty((3, 5, Bn, Psel), np.float32)
    for b in range(Bn):
        r = res.results[b]
        # device layout is [p, m, n] with token t == m*128 + p
        def tok(a):
            return a.transpose(1, 0, 2).reshape(Tp, -1)[:Psel]

        Q[b] = tok(r["q_out"])
        K[b] = tok(r["k_out"])
        V[b] = tok(r["v_out"])
        proj[:, :, b, :] = tok(r["p_out"])[:, :15].T.reshape(3, 5, Psel)
    return Q, K, V, proj, mask
